# revision 15
# baseline (speedup 1.0000x reference)
"""Trainium2 Bass kernel for nn_CumulativeFlattenedLinear (segment_reduce).

Computation: per window of S=64 timesteps, per-timestep C->O linear projection
(weights zero for the first n_discard steps) followed by a causal cumsum within
the window, plus bias.

Strategy (data-parallel over batch, 1 batch element per core), v2:
  - x is DMA'd with partition = 256-element time chunk (1KB contiguous runs).
  - One DVE copy per supertile fuses the (c,w,u,v)->(w,u,c,v) column shuffle
    with an fp32->fp16 cast (2x DVE mode).
  - Per window: 6 PE transposes (fp16) of the sub-block columns into ONE fp16
    PSUM bank; a single ACT copy moves them to SBUF as the matmul stationary.
  - Per sub-block u: a triangular "intra" matmul (n=128) plus a thin "pre"
    matmul (n<=96) that writes the sub-block's totals into the slot-suffix of a
    shared PSUM region; PSUM accumulation across sub-blocks yields the per-slot
    prefix sums directly (no DVE tree adds).
  - DVE adds bias to the prefix region, then one strided combine per window
    writes the (o, t)-ordered output tile; GPSIMD fills the bias-only prefix
    timesteps; output stored with 1KB contiguous runs.
"""
import numpy as np

import concourse.bass as bass
import concourse.tile as tile
from concourse import bacc, mybir
from concourse.bass_utils import run_bass_kernel_spmd

F32 = mybir.dt.float32
F16 = mybir.dt.float16

# problem geometry (asserted against inputs at runtime)
B, C, T, O = 8, 16, 131072, 16
P = 128
CH = 256                 # time-elements per partition per supertile
NST = T // (P * CH)      # 4 supertiles
V = 8                    # sub-block length
NU = 8                   # sub-blocks per window
S = NU * V               # 64
NW = CH // S             # windows per partition = 4

_cache = {}


def _pre_slices(DU):
    """Per-du (psum_off, ncols) for the thin pre matmuls. Slot du' in the
    shared [P, DU*O] pre region accumulates totals of sub-blocks du < du'.
    du=0 writes the whole region (slot 0 zero-weighted) so start=True covers
    it; later du write their slot-suffix."""
    out = []
    for du in range(DU - 1):
        if du == 0:
            out.append((0, DU * O))
        else:
            out.append(((du + 1) * O, (DU - 1 - du) * O))
    return out


def _build_nc(du_count):
    DU = du_count
    assert DU == 6, "start/stop flag layout below assumes DU=6"
    first_u = NU - DU          # first active sub-block
    fill_s = first_u * V       # s < fill_s -> output = bias
    pre_sl = _pre_slices(DU)
    PRE_W = sum(n for _, n in pre_sl)
    # PSUM bank layout for the [P, DU*128 + DU*O] f32 tile (bank = 512 f32):
    #   bank0: tri du=0..3; bank1: tri du=4..5 + pre region.
    # One accumulation group per bank: start=True only on the first writer,
    # stop=True on the last. Program order in bank1: pre0, pre1..4
    # interleaved with tri4, tri5 (tri writes land on fresh columns).
    tri_start = {0: True, 1: False, 2: False, 3: False, 4: False, 5: False}
    tri_stop = {0: False, 1: False, 2: False, 3: True, 4: False, 5: True}

    nc = bacc.Bacc("TRN2", target_bir_lowering=False, debug=False)
    x_d = nc.dram_tensor("x", (C, T), F32, kind="ExternalInput")
    wtri_d = nc.dram_tensor("w_tri", (P, DU * 128), F16, kind="ExternalInput")
    wpre_d = nc.dram_tensor("w_pre", (P, PRE_W), F16, kind="ExternalInput")
    bpre_d = nc.dram_tensor("bias_pre", (P, DU * O), F32, kind="ExternalInput")
    ident_d = nc.dram_tensor("ident", (P, P), F16, kind="ExternalInput")
    bfill_d = nc.dram_tensor("bias_fill", (P, O * fill_s), F32,
                             kind="ExternalInput")
    y_d = nc.dram_tensor("y", (O, T), F32, kind="ExternalOutput")

    xv = x_d.ap().rearrange("c (st p hs) -> st p c hs", st=NST, p=P, hs=CH)
    yv = y_d.ap().rearrange("o (st p hs) -> st p o hs", st=NST, p=P, hs=CH)

    with tile.TileContext(nc) as tc:
        with (
            tc.tile_pool(name="const", bufs=1) as cp,
            tc.tile_pool(name="io", bufs=2) as io,
            tc.tile_pool(name="mid", bufs=3) as mid,
            tc.tile_pool(name="psT", bufs=2, space="PSUM") as psT,
            tc.tile_pool(name="psW", bufs=3, space="PSUM") as psW,
        ):
            # All input DMAs go first on the sync HWDGE ring; alternating tags
            # (2 bufs each -> 4 resident buffers) keep each trigger free of
            # write-after-read waits on earlier shuffles, so the input stream
            # runs back-to-back at line rate.
            HH = CH // 2  # half-supertile time extent per partition
            xins = []
            for st in range(NST):
                xin = io.tile([P, C * CH], F32, name=f"xin{st}",
                              tag=f"xin{st % 2}")
                for h in range(2):
                    nc.sync.dma_start(
                        xin[:].rearrange("p (c hs) -> p c hs", c=C)
                        [:, :, h * HH:(h + 1) * HH],
                        xv[st][:, :, h * HH:(h + 1) * HH],
                    )
                xins.append(xin)

            # constants ride the scalar HWDGE ring (idle until the first
            # output DMA) so they don't delay the input stream
            wtri = cp.tile([P, DU * 128], F16, name="w_tri")
            nc.scalar.dma_start(wtri[:], wtri_d.ap())
            wpre = cp.tile([P, PRE_W], F16, name="w_pre")
            nc.scalar.dma_start(wpre[:], wpre_d.ap())
            bias_pre = cp.tile([P, DU * O], F32, name="bias_pre")
            nc.scalar.dma_start(bias_pre[:], bpre_d.ap())
            ident = cp.tile([P, P], F16, name="ident")
            nc.scalar.dma_start(ident[:], ident_d.ap())
            bfill = cp.tile([P, O * fill_s], F32, name="bfill")
            nc.scalar.dma_start(bfill[:], bfill_d.ap())

            # The bias-only prefix timesteps (s < fill_s) sit at fixed column
            # offsets of the out tiles and are never overwritten by the
            # combine; write them once per round-robin buffer up front and let
            # every supertile's output DMA re-read them.
            for _ in range(2):
                ob = io.tile([P, O * CH], F32, name="out_sb", tag="out")
                dst = ob[:].rearrange(
                    "p (o w s) -> p o w s", o=O, w=NW
                )[:, :, :, 0:fill_s]
                src = bfill[:].rearrange("p (o s) -> p o s", o=O)
                src = src.unsqueeze(2).broadcast_to([P, O, NW, fill_s])
                nc.scalar.copy(dst, src)

            for st in range(NST):
                xin = xins[st]
                shuf = mid.tile([P, NW * DU * 128], F16, name="shuf",
                                tag="shuf")
                out_sb = io.tile([P, O * CH], F32, name="out_sb", tag="out")
                for wdw in range(NW):
                    # fused shuffle + fp16 cast, per half-supertile (DVE 2x) —
                    # gated on the matching half of the input DMA only
                    if wdw % 2 == 0:
                        h = wdw // 2
                        src = xin[:].rearrange(
                            "p (c hh w u v) -> p hh w u c v",
                            c=C, hh=2, w=NW // 2, u=NU, v=V
                        )[:, h, :, first_u:NU]
                        nc.vector.tensor_copy(
                            shuf[:].rearrange(
                                "p (hh w u c v) -> p hh w u c v",
                                hh=2, w=NW // 2, u=DU, c=C, v=V
                            )[:, h],
                            src,
                        )
                    # ---- transposes (PE) into one fp16 psum bank ----
                    ptw = psT.tile([P, DU * 128], F16, name="ptw", tag="ptw")
                    for du in range(DU):
                        nc.tensor.transpose(
                            ptw[:, du * 128:(du + 1) * 128],
                            shuf[:, (wdw * DU + du) * 128:
                                 (wdw * DU + du + 1) * 128],
                            ident[:],
                            tile_position=(0, 0),
                        )
                    ts = mid.tile([P, DU * 128], F16, name="ts", tag="ts")
                    nc.scalar.copy(ts[:], ptw[:])
                    # ---- matmuls: tri (n=128) + thin pre (slot suffix) ----
                    pw = psW.tile([P, DU * 128 + DU * O], F32, name="pw",
                                  tag="pw")
                    woff = 0
                    for du in range(DU):
                        lhs = ts[:, du * 128:(du + 1) * 128]
                        if du < DU - 1:
                            # pre before tri so bank1's group opens with pre0
                            off, n = pre_sl[du]
                            nc.tensor.matmul(
                                pw[:, DU * 128 + off:DU * 128 + off + n],
                                lhs,
                                wpre[:, woff:woff + n],
                                start=(du == 0), stop=False,
                                skip_group_check=True,
                            )
                            woff += n
                        nc.tensor.matmul(
                            pw[:, du * 128:(du + 1) * 128],
                            lhs,
                            wtri[:, du * 128:(du + 1) * 128],
                            start=tri_start[du], stop=tri_stop[du],
                            skip_group_check=True,
                        )
                    # ---- prefix totals + bias (DVE) ----
                    pre_sb = mid.tile([P, DU * O], F32, name="pre_sb",
                                      tag="pre_sb")
                    nc.vector.tensor_add(
                        pre_sb[:], pw[:, DU * 128:DU * 128 + DU * O],
                        bias_pre[:],
                    )
                    # ---- combine: out[(o, s)] = intra + pre_bcast ----
                    out4 = out_sb[:].rearrange(
                        "p (o w u v) -> w p o u v", o=O, w=NW, u=NU, v=V
                    )[wdw, :, :, first_u:NU]
                    in1 = pw[:, 0:DU * 128].rearrange(
                        "p (du v o) -> p o du v", du=DU, v=V, o=O
                    )
                    in2 = pre_sb[:].rearrange("p (du o) -> p du o", du=DU)
                    in2 = in2.transpose([0, 2, 1]).unsqueeze(3)
                    in2 = in2.broadcast_to([P, O, DU, V])
                    nc.vector.tensor_add(out4, in1, in2)
                    # half-supertile output DMA right after its 2 windows
                    if wdw % 2 == 1:
                        h = wdw // 2
                        nc.scalar.dma_start(
                            yv[st][:, :, h * HH:(h + 1) * HH],
                            out_sb[:].rearrange("p (o hs) -> p o hs", o=O)
                            [:, :, h * HH:(h + 1) * HH],
                        )
    nc.compile()
    return nc


def _host_constants(weight, bias, n_discard, n_keep):
    assert n_discard + n_keep == S
    w = weight.reshape(O, C, n_keep).transpose(2, 1, 0)  # (n_keep, C, O)
    w_full = np.concatenate(
        [np.zeros((n_discard, C, O), np.float32), w.astype(np.float32)], axis=0
    )  # (S, C, O)
    act = [u for u in range(NU)
           if np.abs(w_full[u * V:(u + 1) * V]).max() > 0]
    first_u = act[0] if act else NU
    assert act == list(range(first_u, NU))
    DU = len(act)
    vp_idx = np.arange(V)
    tri_blocks = []
    blk_tot = []  # per-du (C*V, O) total-weights
    for u in act:
        blk = w_full[u * V:(u + 1) * V]  # (V, C, O)
        tri = np.zeros((C, V, V, O), np.float32)
        for v in range(V):
            tri[:, vp_idx <= v, v, :] = blk.transpose(1, 0, 2)[:, vp_idx <= v]
        tri_blocks.append(tri.reshape(C * V, V * O))
        blk_tot.append(blk.transpose(1, 0, 2).reshape(C * V, O))
    w_tri = np.concatenate(tri_blocks, axis=1)  # (128, DU*128)
    # thin pre blocks: du writes slots (du', o); slot du' accumulates totals
    # of earlier sub-blocks
    pre_cols = []
    for du, (off, n) in enumerate(_pre_slices(DU)):
        nslots = n // O
        lo_slot = off // O
        blkw = np.zeros((C * V, nslots, O), np.float32)
        for j in range(nslots):
            if lo_slot + j > du:  # slot index du' > du gets tot_du
                blkw[:, j, :] = blk_tot[du]
        pre_cols.append(blkw.reshape(C * V, n))
    w_pre = (np.concatenate(pre_cols, axis=1) if pre_cols
             else np.zeros((C * V, 0), np.float32))
    bias32 = bias.astype(np.float32)
    fill_s = first_u * V
    consts = {
        "w_tri": np.ascontiguousarray(w_tri.astype(np.float16)),
        "w_pre": np.ascontiguousarray(w_pre.astype(np.float16)),
        "bias_pre": np.ascontiguousarray(
            np.tile(bias32, DU)[None, :] * np.ones((P, 1), np.float32)
        ),
        "ident": np.ascontiguousarray(np.eye(P, dtype=np.float16)),
        "bias_fill": np.ascontiguousarray(
            np.tile(bias32[:, None], (1, fill_s)).reshape(1, -1)
            * np.ones((P, 1), np.float32)
        ),
    }
    return consts, DU


def _run(inputs, trace=False):
    x = np.asarray(inputs["x"], dtype=np.float32)
    weight = np.asarray(inputs["weight"], dtype=np.float32)
    bias = np.asarray(inputs["bias"], dtype=np.float32)
    n_discard = int(inputs["n_discard"])
    n_keep = int(inputs["n_keep"])
    assert x.shape == (B, C, T) and weight.shape == (O, C * n_keep)

    consts, DU = _host_constants(weight, bias, n_discard, n_keep)
    key = ("nc", DU)
    if key not in _cache:
        _cache[key] = _build_nc(DU)
    nc = _cache[key]

    in_maps = []
    for b in range(B):
        m = dict(consts)
        m["x"] = np.ascontiguousarray(x[b])
        in_maps.append(m)
    res = run_bass_kernel_spmd(nc, in_maps, list(range(B)), trace=trace)
    y = np.stack([res.results[b]["y"] for b in range(B)], axis=0)
    return y, res


def kernel(**inputs):
    y, _ = _run(inputs, trace=False)
    return y


# revision 20
# speedup vs baseline: 1.0880x; 1.0880x over previous
"""Trainium2 Bass kernel for nn_CumulativeFlattenedLinear (segment_reduce).

Computation: per window of S=64 timesteps, per-timestep C->O linear projection
(weights zero for the first n_discard steps) followed by a causal cumsum within
the window, plus bias.

Strategy (data-parallel over batch, 1 batch element per core), v2:
  - x is DMA'd with partition = 256-element time chunk (1KB contiguous runs).
  - One DVE copy per supertile fuses the (c,w,u,v)->(w,u,c,v) column shuffle
    with an fp32->fp16 cast (2x DVE mode).
  - Per window: 6 PE transposes (fp16) of the sub-block columns into ONE fp16
    PSUM bank; a single ACT copy moves them to SBUF as the matmul stationary.
  - Per sub-block u: a triangular "intra" matmul (n=128) plus a thin "pre"
    matmul (n<=96) that writes the sub-block's totals into the slot-suffix of a
    shared PSUM region; PSUM accumulation across sub-blocks yields the per-slot
    prefix sums directly (no DVE tree adds).
  - DVE adds bias to the prefix region, then one strided combine per window
    writes the (o, t)-ordered output tile; GPSIMD fills the bias-only prefix
    timesteps; output stored with 1KB contiguous runs.
"""
import numpy as np

import concourse.bass as bass
import concourse.tile as tile
from concourse import bacc, mybir
from concourse.bass_utils import run_bass_kernel_spmd

F32 = mybir.dt.float32
F16 = mybir.dt.float16

# problem geometry (asserted against inputs at runtime)
B, C, T, O = 8, 16, 131072, 16
P = 128
CH = 256                 # time-elements per partition per supertile
NST = T // (P * CH)      # 4 supertiles
V = 8                    # sub-block length
NU = 8                   # sub-blocks per window
S = NU * V               # 64
NW = CH // S             # windows per partition = 4

_cache = {}


def _pre_slices(DU):
    """Per-du (psum_off, ncols) for the thin pre matmuls. Slot du' in the
    shared [P, DU*O] pre region accumulates totals of sub-blocks du < du'.
    du=0 writes the whole region (slot 0 zero-weighted) so start=True covers
    it; later du write their slot-suffix."""
    out = []
    for du in range(DU - 1):
        if du == 0:
            out.append((0, DU * O))
        else:
            out.append(((du + 1) * O, (DU - 1 - du) * O))
    return out


def _build_nc(du_count):
    DU = du_count
    assert DU == 6, "start/stop flag layout below assumes DU=6"
    first_u = NU - DU          # first active sub-block
    fill_s = first_u * V       # s < fill_s -> output = bias
    pre_sl = _pre_slices(DU)
    PRE_W = sum(n for _, n in pre_sl)
    # PSUM bank layout for the [P, DU*128 + DU*O] f32 tile (bank = 512 f32):
    #   bank0: tri du=0..3; bank1: tri du=4..5 + pre region.
    # One accumulation group per bank: start=True only on the first writer,
    # stop=True on the last. Program order in bank1: pre0, pre1..4
    # interleaved with tri4, tri5 (tri writes land on fresh columns).
    tri_start = {0: True, 1: False, 2: False, 3: False, 4: False, 5: False}
    tri_stop = {0: False, 1: False, 2: False, 3: True, 4: False, 5: True}

    nc = bacc.Bacc("TRN2", target_bir_lowering=False, debug=False)
    x_d = nc.dram_tensor("x", (C, T), F32, kind="ExternalInput")
    wtri_d = nc.dram_tensor("w_tri", (P, DU * 128), F16, kind="ExternalInput")
    wpre_d = nc.dram_tensor("w_pre", (P, PRE_W), F16, kind="ExternalInput")
    bpre_d = nc.dram_tensor("bias_pre", (P, DU * O), F32, kind="ExternalInput")
    ident_d = nc.dram_tensor("ident", (P, P), F16, kind="ExternalInput")
    y_d = nc.dram_tensor("y", (O, T), F32, kind="ExternalOutput")

    xv = x_d.ap().rearrange("c (st p hs) -> st p c hs", st=NST, p=P, hs=CH)
    yv = y_d.ap().rearrange("o (st p hs) -> st p o hs", st=NST, p=P, hs=CH)

    with tile.TileContext(nc) as tc:
        with (
            tc.tile_pool(name="const", bufs=1) as cp,
            tc.tile_pool(name="io", bufs=2) as io,
            tc.tile_pool(name="mid", bufs=3) as mid,
            tc.tile_pool(name="psT", bufs=2, space="PSUM") as psT,
            tc.tile_pool(name="psW", bufs=3, space="PSUM") as psW,
        ):
            # All input DMAs go first on the sync HWDGE ring; alternating tags
            # (2 bufs each -> 4 resident buffers) keep each trigger free of
            # write-after-read waits on earlier shuffles, so the input stream
            # runs back-to-back at line rate.
            xins = []
            for st in range(NST):
                xin = io.tile([P, C * CH], F32, name=f"xin{st}",
                              tag=f"xin{st % 2}")
                nc.sync.dma_start(
                    xin[:].rearrange("p (c hs) -> p c hs", c=C), xv[st]
                )
                xins.append(xin)

            # constants ride the scalar HWDGE ring (idle until the first
            # output DMA) so they don't delay the input stream
            wtri = cp.tile([P, DU * 128], F16, name="w_tri")
            nc.scalar.dma_start(wtri[:], wtri_d.ap())
            wpre = cp.tile([P, PRE_W], F16, name="w_pre")
            nc.scalar.dma_start(wpre[:], wpre_d.ap())
            bias_pre = cp.tile([P, DU * O], F32, name="bias_pre")
            nc.scalar.dma_start(bias_pre[:], bpre_d.ap())
            ident = cp.tile([P, P], F16, name="ident")
            nc.scalar.dma_start(ident[:], ident_d.ap())

            # The bias-only prefix timesteps (s < fill_s) sit at fixed column
            # offsets of the out tiles and are never overwritten by the
            # combine; write them once per round-robin buffer up front and let
            # every supertile's output DMA re-read them. Slot 0 of bias_pre is
            # exactly bias[o]; broadcast it over (w, s).
            for _ in range(2):
                ob = io.tile([P, O * CH], F32, name="out_sb", tag="out")
                dst = ob[:].rearrange(
                    "p (o w s) -> p o w s", o=O, w=NW
                )[:, :, :, 0:fill_s]
                src = bias_pre[:, 0:O].unsqueeze(2).unsqueeze(3)
                src = src.broadcast_to([P, O, NW, fill_s])
                nc.scalar.copy(dst, src)

            for st in range(NST):
                xin = xins[st]
                shuf = mid.tile([P, NW * DU * 128], F16, name="shuf",
                                tag="shuf")
                out_sb = io.tile([P, O * CH], F32, name="out_sb", tag="out")
                for wdw in range(NW):
                    # fused shuffle + fp16 cast, per half-supertile (DVE 2x) —
                    # gated on the matching half of the input DMA only
                    if wdw % 2 == 0:
                        h = wdw // 2
                        src = xin[:].rearrange(
                            "p (c hh w u v) -> p hh w u c v",
                            c=C, hh=2, w=NW // 2, u=NU, v=V
                        )[:, h, :, first_u:NU]
                        nc.vector.tensor_copy(
                            shuf[:].rearrange(
                                "p (hh w u c v) -> p hh w u c v",
                                hh=2, w=NW // 2, u=DU, c=C, v=V
                            )[:, h],
                            src,
                        )
                    # ---- transposes (PE) into one fp16 psum bank ----
                    ptw = psT.tile([P, DU * 128], F16, name="ptw", tag="ptw")
                    for du in range(DU):
                        nc.tensor.transpose(
                            ptw[:, du * 128:(du + 1) * 128],
                            shuf[:, (wdw * DU + du) * 128:
                                 (wdw * DU + du + 1) * 128],
                            ident[:],
                            tile_position=(0, 0),
                        )
                    ts = mid.tile([P, DU * 128], F16, name="ts", tag="ts")
                    nc.scalar.copy(ts[:], ptw[:])
                    # ---- matmuls: tri (n=128) + thin pre (slot suffix) ----
                    pw = psW.tile([P, DU * 128 + DU * O], F32, name="pw",
                                  tag="pw")
                    woff = 0
                    for du in range(DU):
                        lhs = ts[:, du * 128:(du + 1) * 128]
                        if du < DU - 1:
                            # pre before tri so bank1's group opens with pre0
                            off, n = pre_sl[du]
                            nc.tensor.matmul(
                                pw[:, DU * 128 + off:DU * 128 + off + n],
                                lhs,
                                wpre[:, woff:woff + n],
                                start=(du == 0), stop=False,
                                skip_group_check=True,
                            )
                            woff += n
                        nc.tensor.matmul(
                            pw[:, du * 128:(du + 1) * 128],
                            lhs,
                            wtri[:, du * 128:(du + 1) * 128],
                            start=tri_start[du], stop=tri_stop[du],
                            skip_group_check=True,
                        )
                    # ---- prefix totals + bias (DVE) ----
                    pre_sb = mid.tile([P, DU * O], F32, name="pre_sb",
                                      tag="pre_sb")
                    nc.vector.tensor_add(
                        pre_sb[:], pw[:, DU * 128:DU * 128 + DU * O],
                        bias_pre[:],
                    )
                    # ---- combine: out[(o, s)] = intra + pre_bcast ----
                    out4 = out_sb[:].rearrange(
                        "p (o w u v) -> w p o u v", o=O, w=NW, u=NU, v=V
                    )[wdw, :, :, first_u:NU]
                    in1 = pw[:, 0:DU * 128].rearrange(
                        "p (du v o) -> p o du v", du=DU, v=V, o=O
                    )
                    in2 = pre_sb[:].rearrange("p (du o) -> p du o", du=DU)
                    in2 = in2.transpose([0, 2, 1]).unsqueeze(3)
                    in2 = in2.broadcast_to([P, O, DU, V])
                    nc.vector.tensor_add(out4, in1, in2)
                nc.scalar.dma_start(
                    yv[st], out_sb[:].rearrange("p (o hs) -> p o hs", o=O)
                )
    nc.compile()
    return nc


def _host_constants(weight, bias, n_discard, n_keep):
    assert n_discard + n_keep == S
    w = weight.reshape(O, C, n_keep).transpose(2, 1, 0)  # (n_keep, C, O)
    w_full = np.concatenate(
        [np.zeros((n_discard, C, O), np.float32), w.astype(np.float32)], axis=0
    )  # (S, C, O)
    act = [u for u in range(NU)
           if np.abs(w_full[u * V:(u + 1) * V]).max() > 0]
    first_u = act[0] if act else NU
    assert act == list(range(first_u, NU))
    DU = len(act)
    vp_idx = np.arange(V)
    tri_blocks = []
    blk_tot = []  # per-du (C*V, O) total-weights
    for u in act:
        blk = w_full[u * V:(u + 1) * V]  # (V, C, O)
        tri = np.zeros((C, V, V, O), np.float32)
        for v in range(V):
            tri[:, vp_idx <= v, v, :] = blk.transpose(1, 0, 2)[:, vp_idx <= v]
        tri_blocks.append(tri.reshape(C * V, V * O))
        blk_tot.append(blk.transpose(1, 0, 2).reshape(C * V, O))
    w_tri = np.concatenate(tri_blocks, axis=1)  # (128, DU*128)
    # thin pre blocks: du writes slots (du', o); slot du' accumulates totals
    # of earlier sub-blocks
    pre_cols = []
    for du, (off, n) in enumerate(_pre_slices(DU)):
        nslots = n // O
        lo_slot = off // O
        blkw = np.zeros((C * V, nslots, O), np.float32)
        for j in range(nslots):
            if lo_slot + j > du:  # slot index du' > du gets tot_du
                blkw[:, j, :] = blk_tot[du]
        pre_cols.append(blkw.reshape(C * V, n))
    w_pre = (np.concatenate(pre_cols, axis=1) if pre_cols
             else np.zeros((C * V, 0), np.float32))
    bias32 = bias.astype(np.float32)
    fill_s = first_u * V
    consts = {
        "w_tri": np.ascontiguousarray(w_tri.astype(np.float16)),
        "w_pre": np.ascontiguousarray(w_pre.astype(np.float16)),
        "bias_pre": np.ascontiguousarray(
            np.tile(bias32, DU)[None, :] * np.ones((P, 1), np.float32)
        ),
        "ident": np.ascontiguousarray(np.eye(P, dtype=np.float16)),
    }
    return consts, DU


def _run(inputs, trace=False):
    x = np.asarray(inputs["x"], dtype=np.float32)
    weight = np.asarray(inputs["weight"], dtype=np.float32)
    bias = np.asarray(inputs["bias"], dtype=np.float32)
    n_discard = int(inputs["n_discard"])
    n_keep = int(inputs["n_keep"])
    assert x.shape == (B, C, T) and weight.shape == (O, C * n_keep)

    consts, DU = _host_constants(weight, bias, n_discard, n_keep)
    key = ("nc", DU)
    if key not in _cache:
        _cache[key] = _build_nc(DU)
    nc = _cache[key]

    in_maps = []
    for b in range(B):
        m = dict(consts)
        m["x"] = np.ascontiguousarray(x[b])
        in_maps.append(m)
    res = run_bass_kernel_spmd(nc, in_maps, list(range(B)), trace=trace)
    y = np.stack([res.results[b]["y"] for b in range(B)], axis=0)
    return y, res


def kernel(**inputs):
    y, _ = _run(inputs, trace=False)
    return y
